# revision 40
# baseline (speedup 1.0000x reference)
"""Trainium2 Bass kernel for nn_BiAttn_TFN_hg_2desc_Net (GNN message passing).

Strategy (8 NeuronCores, SPMD single program):
  - Nodes/graphs sharded by graph (64 graphs/core, contiguous node ranges since
    graph_id is sorted). Edges sharded by dst-owner core.
  - L1 aggregates RAW feat (not feat @ W1): agg1 = A @ feat, then the 128->100
    W1 transform runs per 128-node block AFTER aggregation (8x less matmul
    work than transforming all 50k nodes, no t1 table write, and edge gathers
    start immediately from a host-packed feat table).
  - Edge aggregation: edges bucketed host-side by (dst 128-block, src parity);
    dma_gather (1024 idx/call, the single-packet 64-desc/lane ceiling) fetches
    512B rows (2 nodes) from the packed table, idx = src//2; one-hot selectors
    built in bulk (one bf16 is_equal over [128, 32*128] per 32-tile group via
    a broadcast AP); segment-sum by dst via selector matmuls into PSUM.
  - Block epilogue: hm = agg*rdeg -> PE transpose -> h1 = relu(hmT.T @ W1+b1)
    -> t2 = h1 @ gc2_W via a second PE transpose.
  - One AllGather of the tight t2 ([NPAD,32] bf16) across the 8 cores.
  - L2 aggregation: gather DIRECTLY from the allgathered [TOTPAD, 32] table
    with idx = src_pad//4 (256B rows cover 4 nodes; the rhs slice picks
    sub-row src_pad%4), segment-sum as L1, then graph-mean pooling via
    one-hot graph-selector matmuls.
  - Head: bilinear attention + fusion outer-product + 3-layer MLP with
    BatchNorm, computed feature-major; BN batch stats via two tiny AllReduces.
"""

import sys

sys.path.insert(0, "/opt/trn_rl_repo")

import numpy as np
import ml_dtypes

import concourse.bass as bass
import concourse.bacc as bacc
import concourse.tile as tile
from concourse import mybir
from concourse import bass_utils
from concourse.library_config import mlp as _mlp_lib

bass_utils.upload_artifacts = lambda tmpdir: "local://skipped"

P = 128
TGG = 16        # tiles per dma_gather call (2048 idx; needs single_packet=False
                # and a 32KB SWDGE scratch so 129 descs/lane fit the ring)
TGS = 32        # tiles per bulk selector build
BN_EPS = 1e-5

F32 = mybir.dt.float32
BF16 = mybir.dt.bfloat16
I16 = mybir.dt.int16

BF = ml_dtypes.bfloat16


# ----------------------------------------------------------------------------
# Host-side planning
# ----------------------------------------------------------------------------

def _wrap_idx(flat_idx):
    """[NI] int -> [128, NI//16] int16 in the dma_gather wrapped layout
    (idx i at [i % 16, i // 16], tiled x8 down the partitions)."""
    a = np.asarray(flat_idx, np.int16).reshape(-1, 16).T      # [16, NI/16]
    return np.tile(a, (8, 1))                                  # [128, NI/16]


def plan(inputs, nc_cores, dims):
    """Host preprocessing. Returns (meta, per_core_inputs)."""
    N = dims["N"]; E = dims["E"]; B = dims["B"]
    DIM_IN = dims["DIM_IN"]; GC1 = dims["GC1"]; DG = dims["DG"]
    D2 = dims["D2"]; DH = dims["DH"]; H1 = dims["H1"]; H2 = dims["H2"]
    NC = nc_cores
    GPC = B // NC

    feat = np.asarray(inputs["feat"], np.float32)
    src = np.asarray(inputs["src"], np.int64)
    dst = np.asarray(inputs["dst"], np.int64)
    gid = np.asarray(inputs["graph_id"], np.int64)

    # --- core node/graph ranges (graph-aligned) ---
    bounds = np.searchsorted(gid, np.arange(0, B + 1, GPC))
    g_start, g_end = bounds[:-1], bounds[1:]
    nodes_c = g_end - g_start
    B_blk = int(np.ceil(nodes_c.max() / P))
    NPAD = B_blk * P
    TOTPAD = NC * NPAD
    NROWS = int(np.ceil(N / P)) * P            # feat rows (node space, 128-pad)
    assert NROWS % 2 == 0 and NROWS // 2 < 32768
    assert TOTPAD % 4 == 0 and TOTPAD // 4 < 32768

    # --- degrees / counts ---
    deg = np.bincount(dst, minlength=N).astype(np.float32)
    rdeg_full = 1.0 / np.maximum(deg, 1.0)
    cnt = np.bincount(gid, minlength=B).astype(np.float32)
    rcnt_full = (1.0 / np.maximum(cnt, 1.0)).astype(np.float32)

    # --- edge assignment ---
    core_of_dst = np.searchsorted(g_end - 1, dst)          # g_start <= dst < g_end
    core_of_src = np.searchsorted(g_end - 1, src)
    src_pad = core_of_src * NPAD + (src - g_start[core_of_src])

    # L1 subbuckets: (dst block, src % 2); L2: (dst block, src_pad % 4)
    per_core_edges = []
    cnt1 = np.zeros((NC, B_blk, 2), np.int64)
    cnt2 = np.zeros((NC, B_blk, 4), np.int64)
    for c in range(NC):
        m = core_of_dst == c
        e_src, e_dst, e_srcpad = src[m], dst[m], src_pad[m]
        drel = e_dst - g_start[c]
        blk = drel // P
        drel_in = (drel % P).astype(np.float32)
        i1 = e_src // 2
        p1 = e_src % 2
        i2 = e_srcpad // 4
        q2 = e_srcpad % 4
        buckets = {}
        for b in range(B_blk):
            mb = blk == b
            for p in range(2):
                mm = mb & (p1 == p)
                buckets[("L1", b, p)] = (i1[mm], drel_in[mm])
                cnt1[c, b, p] = mm.sum()
            for q in range(4):
                mm = mb & (q2 == q)
                buckets[("L2", b, q)] = (i2[mm], drel_in[mm])
                cnt2[c, b, q] = mm.sum()
        per_core_edges.append(buckets)

    def _layer_plan(cnt):
        """Dense layout shared across cores: run (b, s) sized to the
        max-over-cores edge count; tiles are 128-chunks of the flat
        concatenation; one selector-matmul per (tile, run) overlap."""
        nsub = cnt.shape[2]
        Lmax = cnt.max(axis=0)                       # [B_blk, nsub]
        for b in range(B_blk):
            if Lmax[b].sum() == 0:
                Lmax[b, 0] = 1                       # keep psum group alive
        off = np.zeros((B_blk, nsub), np.int64)
        tot = 0
        for b in range(B_blk):
            for s in range(nsub):
                off[b, s] = tot
                tot += int(Lmax[b, s])
        NT = -(-tot // P)
        ML = []                                       # (block, tile, sub, selcol)
        Kb = [0] * B_blk
        selc = 0
        for b in range(B_blk):
            for s in range(nsub):
                a0 = int(off[b, s]); ln = int(Lmax[b, s])
                if ln == 0:
                    continue
                for t in range(a0 // P, (a0 + ln - 1) // P + 1):
                    ML.append((b, t, s, selc))
                    Kb[b] += 1
                    selc += 1
        return Lmax, off, tot, NT, tuple(ML), tuple(Kb), selc

    Lmax1, off1, tot1, NT1, ML1, Kb1, NSEL1 = _layer_plan(cnt1)
    Lmax2, off2, tot2, NT2, ML2, Kb2, NSEL2 = _layer_plan(cnt2)
    NG1 = -(-NT1 // TGG); NG2 = -(-NT2 // TGG)
    NSELP1 = -(-NSEL1 // TGS) * TGS
    NSELP2 = -(-NSEL2 // TGS) * TGS

    import os
    meta = dict(
        PH=int(os.environ.get("K_PHASES", "9")),
        NC=NC, B=B, GPC=GPC, B_blk=B_blk, NPAD=NPAD, TOTPAD=TOTPAD,
        NROWS=NROWS,
        ML1=ML1, ML2=ML2, Kb1=Kb1, Kb2=Kb2,
        NT1=NT1, NT2=NT2, NG1=NG1, NG2=NG2,
        NSEL1=NSEL1, NSEL2=NSEL2, NSELP1=NSELP1, NSELP2=NSELP2,
        DIM_IN=DIM_IN, GC1=GC1, DG=DG, D2=D2, DH=DH, H1=H1, H2=H2,
    )

    # --- shared (replicated) tensors ---
    featP = np.zeros((NROWS // 2, 2 * DIM_IN), BF)       # 512B rows, 2 nodes
    featP[:, :] = np.pad(feat, ((0, NROWS - N), (0, 0))).astype(BF).reshape(
        NROWS // 2, 2 * DIM_IN)
    w1 = np.asarray(inputs["gc1_W"], np.float32).astype(BF)          # [128,100]
    w2 = np.asarray(inputs["gc2_W"], np.float32).astype(BF)          # [100,20]
    iota = np.tile(np.arange(P, dtype=np.float32), (P, 1))
    iotaG = np.tile(np.arange(P, dtype=np.float32), (P, TGS)).astype(BF)
    ident = np.eye(P, dtype=np.float32)
    identb = np.eye(P, dtype=np.float32).astype(BF)
    b1b = np.tile(np.asarray(inputs["gc1_b"], np.float32), (P, 1))   # [128,100]
    b2b = np.tile(np.asarray(inputs["gc2_b"], np.float32), (P, 1))   # [128,20]

    pg_W = np.asarray(inputs["pg_W"], np.float32); pg_b = np.asarray(inputs["pg_b"], np.float32)
    p2_W = np.asarray(inputs["p2_W"], np.float32); p2_b = np.asarray(inputs["p2_b"], np.float32)
    W2m = np.asarray(inputs["W2"], np.float32)
    w2eff = np.concatenate([pg_W, pg_b[None, :]], 0) @ W2m            # [21, 64]
    # d1 columns reordered to [1 | a*desc2d] so the ones-column sits at
    # partition 0 of chunk 0 (engines need aligned start partitions).
    p2w_aug = np.concatenate([p2_b[None, :], p2_W], 0)                # [201, 64]
    # p2w in 128-row chunks (chunk 1 zero-padded to 128)
    p2wp = np.zeros((P, 2, DH), np.float32)
    p2wp[:, 0] = p2w_aug[:P]
    p2wp[:D2 + 1 - P, 1] = p2w_aug[P:]
    # fc1_W re-laid for the feature-major fusion: fusion row r = i*(D2+1)+j
    # (j in the reordered space: j=0 -> original 200 (ones), j>0 -> j-1)
    # maps to padded row i*256 + j -> chunk c = 2i + j//128, partition j%128.
    NCH = 2 * (DG + 1)
    fc1wp = np.zeros((P, NCH, H1), np.float32)
    fc1W_r = np.asarray(inputs["fc1_W"], np.float32)
    jperm = np.concatenate([[D2], np.arange(D2)])      # new j -> original j
    for i in range(DG + 1):
        rows = fc1W_r[i * (D2 + 1) + jperm]            # [201, H1] reordered
        for h in range(2):
            jlo = h * P
            jhi = min((h + 1) * P, D2 + 1)
            fc1wp[:jhi - jlo, 2 * i + h] = rows[jlo:jhi]
    fc2w = np.asarray(inputs["fc2_W"], np.float32)
    fc3w = np.asarray(inputs["fc3_W"], np.float32)
    fc3b_r = np.asarray(inputs["fc3_b"], np.float32)
    bn1g = np.asarray(inputs["bn1_g"], np.float32)[:, None]
    bn1b = np.asarray(inputs["bn1_b"], np.float32)[:, None]
    bn2g = np.asarray(inputs["bn2_g"], np.float32)[:, None]
    bn2b = np.asarray(inputs["bn2_b"], np.float32)[:, None]
    # fc biases ride along as [H,1] columns added before BN.
    # BN(x+c) absorbs additive consts into the mean, so fc1_b/fc2_b cancel
    # entirely; fc3_b survives.
    meta["NCH"] = NCH
    desc2d = np.asarray(inputs["desc_2d"], np.float32)                # [B, 200]

    def _flat_plan(buckets, L, Lmax, off, NT, ML, NSELP):
        """Emit flat idx array (dense runs) and per-matmul selector dr
        columns."""
        nsub = Lmax.shape[1]
        idx_flat = np.zeros(NT * P, np.int64)
        occ_dr = np.full(NT * P, 255.0, np.float32)
        for b in range(B_blk):
            for s in range(nsub):
                ii, dd = buckets[(L, b, s)]
                a0 = int(off[b, s])
                idx_flat[a0:a0 + len(ii)] = ii
                occ_dr[a0:a0 + len(ii)] = dd
        drA = np.full(NSELP * P, 255.0, np.float32)
        for (b, t, s, k) in ML:
            a0 = int(off[b, s]); a1 = a0 + int(Lmax[b, s])
            lo = max(t * P, a0); hi = min((t + 1) * P, a1)
            drA[k * P + (lo - t * P):k * P + (hi - t * P)] = occ_dr[lo:hi]
        return idx_flat, drA

    per_core = []
    for c in range(NC):
        buckets = per_core_edges[c]
        i1f, d1f = _flat_plan(buckets, "L1", Lmax1, off1, NT1, ML1, NSELP1)
        i2f, d2f = _flat_plan(buckets, "L2", Lmax2, off2, NT2, ML2, NSELP2)
        NI = TGG * P
        i1f = np.pad(i1f, (0, NG1 * NI - len(i1f)))
        i2f = np.pad(i2f, (0, NG2 * NI - len(i2f)))
        idx1 = np.zeros((NG1, P, NI // 16), np.int16)
        idx2 = np.zeros((NG2, P, NI // 16), np.int16)
        for g in range(NG1):
            idx1[g] = _wrap_idx(i1f[g * NI:(g + 1) * NI])
        for g in range(NG2):
            idx2[g] = _wrap_idx(i2f[g * NI:(g + 1) * NI])
        dr1 = d1f.reshape(NSELP1, P).T.astype(BF).copy()    # [128, NSELP1]
        dr2 = d2f.reshape(NSELP2, P).T.astype(BF).copy()    # [128, NSELP2]

        nloc = int(nodes_c[c])
        rdeg = np.ones((B_blk * P,), np.float32)
        rdeg[:nloc] = rdeg_full[g_start[c]:g_end[c]]
        gidrel = np.full((B_blk * P,), 255.0, np.float32)
        gidrel[:nloc] = (gid[g_start[c]:g_end[c]] - c * GPC).astype(np.float32)
        rcnt = rcnt_full[c * GPC:(c + 1) * GPC][:, None]              # [GPC,1]
        d2c = desc2d[c * GPC:(c + 1) * GPC]                            # [GPC,200]
        d2T_aug = np.concatenate([np.ones((1, GPC), np.float32), d2c.T], 0)  # [201,GPC]
        d2Tp = np.zeros((P, 2, GPC), np.float32)
        d2Tp[:, 0] = d2T_aug[:P]
        d2Tp[:D2 + 1 - P, 1] = d2T_aug[P:]
        per_core.append({
            "featP": featP, "w1": w1, "w2": w2, "iota": iota, "iotaG": iotaG,
            "ident": ident, "identb": identb, "b1b": b1b, "b2b": b2b,
            "idx1": idx1, "dr1": dr1, "idx2": idx2, "dr2": dr2,
            "rdeg": rdeg.reshape(B_blk, P).T.copy(),      # [128, B_blk]
            "gidrel": gidrel.reshape(B_blk, P).T.copy(),  # [128, B_blk]
            "rcnt": rcnt, "d2gm": d2c, "d2Tp": d2Tp,
            "w2eff": w2eff, "p2wp": p2wp,
            "fc1wp": fc1wp, "fc2w": fc2w, "fc3w": fc3w,
            "fc3b": np.array([[float(fc3b_r[0])]], np.float32),
            "bn1g": bn1g, "bn1b": bn1b, "bn2g": bn2g, "bn2b": bn2b,
        })
    return meta, per_core


# ----------------------------------------------------------------------------
# Device program
# ----------------------------------------------------------------------------

def build(meta):
    NC = meta["NC"]; B = meta["B"]; GPC = meta["GPC"]; B_blk = meta["B_blk"]
    NPAD = meta["NPAD"]; TOTPAD = meta["TOTPAD"]
    NROWS = meta["NROWS"]
    ML1 = meta["ML1"]; ML2 = meta["ML2"]; Kb1 = meta["Kb1"]; Kb2 = meta["Kb2"]
    NT1 = meta["NT1"]; NT2 = meta["NT2"]
    NG1 = meta["NG1"]; NG2 = meta["NG2"]
    NSELP1 = meta["NSELP1"]; NSELP2 = meta["NSELP2"]
    DIM_IN = meta["DIM_IN"]; GC1 = meta["GC1"]; DG = meta["DG"]; D2 = meta["D2"]
    DH = meta["DH"]
    H1 = meta["H1"]; H2 = meta["H2"]; NCH = meta["NCH"]; PH = meta["PH"]
    EQ = mybir.AluOpType.is_equal
    MUL = mybir.AluOpType.mult
    ADD = mybir.AluOpType.add
    SUB = mybir.AluOpType.subtract
    AF = mybir.ActivationFunctionType
    NI = TGG * P

    nc = bacc.Bacc("TRN2", target_bir_lowering=False, debug=False, num_devices=NC,
                   num_swdge_queues=2, dynamic_dma_scratch_size=32768)

    def din(name, shape, dt):
        return nc.dram_tensor(name, shape, dt, kind="ExternalInput").ap()

    featP_d = din("featP", [NROWS // 2, 2 * DIM_IN], BF16)
    w1_d = din("w1", [DIM_IN, GC1], BF16)
    w2_d = din("w2", [GC1, DG], BF16)
    iota_d = din("iota", [P, P], F32)
    iotaG_d = din("iotaG", [P, TGS * P], BF16)
    ident_d = din("ident", [P, P], F32)
    identb_d = din("identb", [P, P], BF16)
    b1b_d = din("b1b", [P, GC1], F32)
    b2b_d = din("b2b", [P, DG], F32)
    idx1_d = din("idx1", [NG1, P, NI // 16], I16)
    dr1_d = din("dr1", [P, NSELP1], BF16)
    idx2_d = din("idx2", [NG2, P, NI // 16], I16)
    dr2_d = din("dr2", [P, NSELP2], BF16)
    rdeg_d = din("rdeg", [P, B_blk], F32)
    gidrel_d = din("gidrel", [P, B_blk], F32)
    rcnt_d = din("rcnt", [GPC, 1], F32)
    d2gm_d = din("d2gm", [GPC, D2], F32)
    d2Tp_d = din("d2Tp", [P, 2, GPC], F32)
    w2eff_d = din("w2eff", [DG + 1, 64], F32)
    p2wp_d = din("p2wp", [P, 2, DH], F32)
    fc1wp_d = din("fc1wp", [P, NCH, H1], F32)
    fc2w_d = din("fc2w", [H1, H2], F32)
    fc3w_d = din("fc3w", [H2, 1], F32)
    fc3b_d = din("fc3b", [1, 1], F32)
    bn1g_d = din("bn1g", [H1, 1], F32)
    bn1b_d = din("bn1b", [H1, 1], F32)
    bn2g_d = din("bn2g", [H2, 1], F32)
    bn2b_d = din("bn2b", [H2, 1], F32)

    t2sh_d = nc.dram_tensor("t2shard", [NPAD, 32], BF16).ap()
    t2full_d = nc.dram_tensor("t2full", [TOTPAD, 32], BF16, addr_space="Shared").ap()
    hgrt_d = nc.dram_tensor("hgrt", [1, (DG + 1) * GPC], F32).ap()
    art_d = nc.dram_tensor("art", [1, GPC], F32).ap()
    bn1i_d = nc.dram_tensor("bn1i", [H1, 2], F32).ap()
    bn1o_d = nc.dram_tensor("bn1o", [H1, 2], F32, addr_space="Shared").ap()
    bn2i_d = nc.dram_tensor("bn2i", [H2, 2], F32).ap()
    bn2o_d = nc.dram_tensor("bn2o", [H2, 2], F32, addr_space="Shared").ap()
    out_d = nc.dram_tensor("out", [1, GPC], F32, kind="ExternalOutput").ap()

    groups = [list(range(NC))]

    class _SkipRest(Exception):
        pass

    with tile.TileContext(nc) as tc:
        from contextlib import ExitStack
        with ExitStack() as ctx:
          try:
            cp = ctx.enter_context(tc.tile_pool(name="consts", bufs=1))
            pp_t1 = ctx.enter_context(tc.tile_pool(name="p_t1", bufs=1, space="PSUM"))
            sb_t1 = ctx.enter_context(tc.tile_pool(name="sb_t1", bufs=4))
            ip = ctx.enter_context(tc.tile_pool(name="idx", bufs=4))
            payp = ctx.enter_context(tc.tile_pool(name="pay", bufs=4))
            selp = ctx.enter_context(tc.tile_pool(name="sel", bufs=2))
            drp = ctx.enter_context(tc.tile_pool(name="dr", bufs=2))
            gsp = ctx.enter_context(tc.tile_pool(name="gsel", bufs=4))
            pp_agg = ctx.enter_context(tc.tile_pool(name="p_agg", bufs=2, space="PSUM"))
            pp_tr = ctx.enter_context(tc.tile_pool(name="p_tr", bufs=1, space="PSUM"))
            pp_t2 = ctx.enter_context(tc.tile_pool(name="p_t2", bufs=1, space="PSUM"))
            hpool = ctx.enter_context(tc.tile_pool(name="hwork", bufs=3))
            pp_hg = ctx.enter_context(tc.tile_pool(name="p_hg", bufs=1, space="PSUM"))
            hd = ctx.enter_context(tc.tile_pool(name="head", bufs=1))

            nc.gpsimd.load_library(_mlp_lib)

            # ---- constants ----
            iota_t = cp.tile([P, P], F32); nc.sync.dma_start(iota_t[:], iota_d[:])
            iotaG_t = cp.tile([P, TGS, P], BF16)
            nc.sync.dma_start(iotaG_t[:], iotaG_d[:].rearrange("p (g e) -> p g e", g=TGS))
            zcol = cp.tile([P, 1], F32); nc.vector.memset(zcol[:], 0.0)
            ident_t = cp.tile([P, P], F32); nc.sync.dma_start(ident_t[:], ident_d[:])
            identb_t = cp.tile([P, P], BF16); nc.sync.dma_start(identb_t[:], identb_d[:])
            w1_t = cp.tile([DIM_IN, GC1], BF16); nc.sync.dma_start(w1_t[:], w1_d[:])
            w2_t = cp.tile([GC1, DG], BF16); nc.sync.dma_start(w2_t[:], w2_d[:])
            b1b_t = cp.tile([P, GC1], F32); nc.sync.dma_start(b1b_t[:], b1b_d[:])
            b2b_t = cp.tile([P, DG], F32); nc.sync.dma_start(b2b_t[:], b2b_d[:])
            rdeg_t = cp.tile([P, B_blk], F32); nc.sync.dma_start(rdeg_t[:], rdeg_d[:])
            gidr_t = cp.tile([P, B_blk], F32); nc.sync.dma_start(gidr_t[:], gidrel_d[:])

            # ================= shared edge-layer builder =====================
            def edge_layer(NT, NG, NSELP, ML, Kb, idx_dram, dr_dram, tab_ap,
                           elem, sub_off, ncols, out_block, paytag):
                """Gather calls (TGG tiles, dense) + bulk selectors (TGS
                columns), then segment-sum following the host matmul list."""
                pay_tiles = []
                sel_tiles = []
                for g in range(NG):
                    gt = min(TGG, NT - g * TGG)
                    ni = gt * P
                    ix = ip.tile([P, NI // 16], I16, tag="ix" + paytag)
                    nc.sync.dma_start(ix[:, :ni // 16], idx_dram[g, :, :ni // 16])
                    pay = payp.tile([P, TGG, elem], BF16, tag=paytag)
                    # alternate SWDGE queues: each queue's descriptor
                    # generation runs on its own Q7 core pair (cpu_id/2 ==
                    # queue_num), letting consecutive gathers overlap.
                    nc.gpsimd.dma_gather(pay[:, :gt, :], tab_ap, ix[:, :ni // 16],
                                         ni, ni, elem, queue_num=g % 2,
                                         single_packet=False)
                    pay_tiles.append(pay)
                for s in range(NSELP // TGS):
                    drG = drp.tile([P, TGS, 1], BF16, tag="dr" + paytag)
                    nc.sync.dma_start(drG[:, :, 0],
                                      dr_dram[:, s * TGS:(s + 1) * TGS])
                    selG = selp.tile([P, TGS, P], BF16, tag="sel" + paytag)
                    nc.vector.tensor_tensor(
                        out=selG[:], in0=iotaG_t[:],
                        in1=drG[:, :, :].broadcast_to([P, TGS, P]), op=EQ)
                    sel_tiles.append(selG)

                cur_b = -1
                ps = None
                ki = 0
                for (b, t, s, k) in ML:
                    if b != cur_b:
                        cur_b = b
                        ps = pp_agg.tile([P, ncols], F32, tag="agg")
                        ki = 0
                    g, off = divmod(t, TGG)
                    sg, soff = divmod(k, TGS)
                    nc.tensor.matmul(
                        ps[:],
                        lhsT=sel_tiles[sg][:, soff, :],
                        rhs=pay_tiles[g][:, off, sub_off * s:sub_off * s + ncols],
                        start=(ki == 0), stop=(ki == Kb[b] - 1))
                    ki += 1
                    if ki == Kb[b]:
                        out_block(b, ps)

            # ---- L1 ----
            def l1_out(b, ps):
                # hm = agg * rdeg (bf16), nodes on partitions
                hm = hpool.tile([P, DIM_IN], BF16, tag="hm")
                nc.vector.tensor_scalar(out=hm[:], in0=ps[:],
                                        scalar1=rdeg_t[:, b:b + 1], scalar2=None,
                                        op0=MUL)
                tp = pp_tr.tile([P, P], BF16, tag="trp")
                nc.tensor.transpose(tp[:], hm[:], identb_t[:])
                hmT = hpool.tile([P, P], BF16, tag="hmT")
                nc.vector.tensor_copy(hmT[:], tp[:])
                h1p = pp_tr.tile([P, GC1], F32, tag="h1p")
                nc.tensor.matmul(h1p[:], lhsT=hmT[:], rhs=w1_t[:], start=True, stop=True)
                h1 = hpool.tile([P, GC1], F32, tag="h1")
                nc.vector.tensor_tensor(out=h1[:], in0=h1p[:], in1=b1b_t[:], op=ADD)
                nc.scalar.activation(out=h1[:], in_=h1[:], func=AF.Relu, bias=zcol[:, :1])
                tp2 = pp_tr.tile([GC1, P], F32, tag="trp2")
                nc.tensor.transpose(tp2[:], h1[:], ident_t[:])
                h1T = hpool.tile([GC1, P], BF16, tag="h1T")
                nc.vector.tensor_copy(h1T[:], tp2[:])
                t2p = pp_t2.tile([P, DG], F32, tag="t2p")
                nc.tensor.matmul(t2p[:], lhsT=h1T[:], rhs=w2_t[:], start=True, stop=True)
                t2s = sb_t1.tile([P, 32], BF16, tag="t2s")
                nc.vector.tensor_copy(t2s[:, :DG], t2p[:])
                nc.vector.memset(t2s[:, DG:], 0.0)
                nc.sync.dma_start(t2sh_d[b * P:(b + 1) * P, :], t2s[:])

            _sc2 = nc.enter_named_scope("ph2_L1", False)
            if PH >= 2:
                edge_layer(NT1, NG1, NSELP1, ML1, Kb1, idx1_d, dr1_d,
                           featP_d[:], 2 * DIM_IN, DIM_IN, DIM_IN, l1_out,
                           "pay1")
            nc.leave_named_scope("ph2_L1", _sc2[0], False)

            # ---- AllGather t2 ----
            _sc3 = nc.enter_named_scope("ph3_ag", False)
            if PH >= 3:
              nc.gpsimd.collective_compute(
                "AllGather", mybir.AluOpType.bypass, replica_groups=groups,
                ins=[t2sh_d[:].opt()], outs=[t2full_d[:].opt()])
            nc.leave_named_scope("ph3_ag", _sc3[0], False)

            # ---- L2 + pooling ----
            do_l2 = PH >= 5
            do_head = PH >= 6
            hg_ps = pp_hg.tile([GPC, DG], F32, tag="hgps")

            def l2_out(b, ps):
                h2t = hpool.tile([P, DG], F32, tag="h2")
                nc.vector.tensor_scalar(out=h2t[:], in0=ps[:],
                                        scalar1=rdeg_t[:, b:b + 1], scalar2=None,
                                        op0=MUL)
                nc.vector.tensor_tensor(out=h2t[:], in0=h2t[:], in1=b2b_t[:], op=ADD)
                nc.scalar.activation(out=h2t[:], in_=h2t[:], func=AF.Relu, bias=zcol[:P, :1])
                selg = gsp.tile([P, GPC], F32, tag="selg")
                nc.vector.tensor_scalar(out=selg[:], in0=iota_t[:, :GPC],
                                        scalar1=gidr_t[:, b:b + 1], scalar2=None,
                                        op0=EQ)
                nc.tensor.matmul(hg_ps[:], lhsT=selg[:], rhs=h2t[:],
                                 start=(b == 0), stop=(b == B_blk - 1),
                                 skip_group_check=True)

            _sc5 = nc.enter_named_scope("ph5_L2", False)
            if do_l2:
                t2view = t2full_d[:].rearrange("(r q) e -> r (q e)", q=4)
                edge_layer(NT2, NG2, NSELP2, ML2, Kb2, idx2_d, dr2_d,
                           t2view, P, 32, DG, l2_out, "pay2")
            nc.leave_named_scope("ph5_L2", _sc5[0], False)

            # ================= Head ==========================================
            if not do_head:
                raise _SkipRest()
            _sc6 = nc.enter_named_scope("ph6_head", False)
            rcnt_t = hd.tile([GPC, 1], F32); nc.sync.dma_start(rcnt_t[:], rcnt_d[:])
            d2gm_t = hd.tile([GPC, D2], F32); nc.sync.dma_start(d2gm_t[:], d2gm_d[:])
            d2Tp_t = hd.tile([P, 2, GPC], F32); nc.sync.dma_start(d2Tp_t[:], d2Tp_d[:])
            w2e_t = hd.tile([DG + 1, 64], F32); nc.sync.dma_start(w2e_t[:], w2eff_d[:])
            p2wp_t = hd.tile([P, 2, DH], F32); nc.sync.dma_start(p2wp_t[:], p2wp_d[:])
            fc1wp_t = hd.tile([P, NCH, H1], F32)
            nc.sync.dma_start(fc1wp_t[:], fc1wp_d[:])
            ones1 = hd.tile([1, P], F32); nc.vector.memset(ones1[:], 1.0)
            fc2w_t = hd.tile([H1, H2], F32); nc.sync.dma_start(fc2w_t[:], fc2w_d[:])
            fc3w_t = hd.tile([H2, 1], F32); nc.sync.dma_start(fc3w_t[:], fc3w_d[:])
            fc3b_t = hd.tile([1, 1], F32); nc.sync.dma_start(fc3b_t[:], fc3b_d[:])
            bn1g_t = hd.tile([H1, 1], F32); nc.sync.dma_start(bn1g_t[:], bn1g_d[:])
            bn1b_t = hd.tile([H1, 1], F32); nc.sync.dma_start(bn1b_t[:], bn1b_d[:])
            bn2g_t = hd.tile([H2, 1], F32); nc.sync.dma_start(bn2g_t[:], bn2g_d[:])
            bn2b_t = hd.tile([H2, 1], F32); nc.sync.dma_start(bn2b_t[:], bn2b_d[:])
            if PH == 60:
                raise _SkipRest()

            # hg1 = [hg | 1]
            hg1 = hd.tile([GPC, DG + 1], F32)
            nc.vector.tensor_scalar(out=hg1[:, :DG], in0=hg_ps[:], scalar1=rcnt_t[:, :1],
                                    scalar2=None, op0=MUL)
            nc.vector.memset(hg1[:, DG:DG + 1], 1.0)
            # hgT
            tp2 = pp_tr.tile([DG + 1, GPC], F32, tag="trp")
            nc.tensor.transpose(tp2[:], hg1[:], ident_t[:GPC, :GPC])
            hgT = hd.tile([DG + 1, GPC], F32)
            nc.vector.tensor_copy(hgT[:], tp2[:])
            # round-trip hgT through DRAM to get it on one partition row
            nc.sync.dma_start(
                hgrt_d[:].rearrange("o (i g) -> (o i) g", i=DG + 1), hgT[:])
            hgflat = hd.tile([1, (DG + 1) * GPC], F32)
            nc.sync.dma_start(hgflat[:], hgrt_d[:])
            # h_gm, h_d (graph-major [GPC, 64])
            hgm_ps = pp_t1.tile([GPC, 64], F32, tag="t1ps")
            nc.tensor.matmul(hgm_ps[:], lhsT=hgT[:], rhs=w2e_t[:], start=True, stop=True)
            hgm_sb = hd.tile([GPC, 64], F32)
            nc.vector.tensor_copy(hgm_sb[:], hgm_ps[:])
            hdm_ps = pp_t1.tile([GPC, 64], F32, tag="t1ps")
            nc.tensor.matmul(hdm_ps[:], lhsT=d2Tp_t[:, 0, :], rhs=p2wp_t[:, 0, :],
                             start=True, stop=False)
            nc.tensor.matmul(hdm_ps[:], lhsT=d2Tp_t[:, 1, :], rhs=p2wp_t[:, 1, :],
                             start=False, stop=True)
            junk = hd.tile([GPC, 64], F32)
            s_t = hd.tile([GPC, 1], F32)
            nc.vector.tensor_tensor(out=junk[:], in0=hgm_sb[:], in1=hdm_ps[:], op=MUL)
            nc.vector.reduce_sum(out=s_t[:], in_=junk[:], axis=mybir.AxisListType.X)
            a_t = hd.tile([GPC, 1], F32)
            nc.scalar.activation(out=a_t[:], in_=s_t[:], func=AF.Sigmoid, bias=zcol[:GPC, :1])
            if PH == 61:
                raise _SkipRest()
            # broadcast a over partitions: DRAM round-trip + ones matmul
            nc.sync.dma_start(art_d[:].rearrange("o (g u) -> (o g) u", u=1), a_t[:])
            aflat = hd.tile([1, GPC], F32)
            nc.sync.dma_start(aflat[:], art_d[:])
            ab_ps = pp_tr.tile([P, GPC], F32, tag="trp")
            nc.tensor.matmul(ab_ps[:], lhsT=ones1[:], rhs=aflat[:],
                             start=True, stop=True)
            aB = hd.tile([P, GPC], F32)
            nc.vector.tensor_copy(aB[:], ab_ps[:])
            # d1Tp = d2Tp * a (ones row restored afterwards)
            d1Tp = hd.tile([P, 2, GPC], F32)
            nc.vector.tensor_tensor(
                out=d1Tp[:], in0=d2Tp_t[:],
                in1=aB[:].rearrange("p (o g) -> p o g", o=1).broadcast_to([P, 2, GPC]),
                op=MUL)
            nc.vector.memset(d1Tp[0:1, 0, :], 1.0)
            # hg1B[p, i, g] = hg1[g, i] replicated across partitions
            # (3 matmul chunks of 7*GPC cols to stay within one PSUM bank)
            hg1B = hd.tile([P, DG + 1, GPC], F32)
            CW = 7 * GPC
            for kk in range(3):
                hgb_ps = pp_tr.tile([P, CW], F32, tag="h1p")
                nc.tensor.matmul(hgb_ps[:], lhsT=ones1[:],
                                 rhs=hgflat[:, kk * CW:(kk + 1) * CW],
                                 start=True, stop=True)
                nc.vector.tensor_copy(
                    hg1B[:, kk * 7:(kk + 1) * 7, :],
                    hgb_ps[:].rearrange("p (i g) -> p i g", i=7))
            # fusT[p, i, h, g] = hg1B[p, i, g] * d1Tp[p, h, g]
            fusT = hd.tile([P, DG + 1, 2, GPC], F32)
            nc.vector.tensor_tensor(
                out=fusT[:],
                in0=hg1B[:].rearrange("p i (o g) -> p i o g", o=1)
                    .broadcast_to([P, DG + 1, 2, GPC]),
                in1=d1Tp[:].rearrange("p (o h) g -> p o h g", o=1)
                    .broadcast_to([P, DG + 1, 2, GPC]),
                op=MUL)
            if PH == 62:
                raise _SkipRest()
            # fc1 (feature-major out [H1, GPC])
            fusTv = fusT[:].rearrange("p i h g -> p (i h) g")
            fc1_ps = pp_t1.tile([H1, GPC], F32, tag="t1ps")
            for kt in range(NCH):
                nc.tensor.matmul(fc1_ps[:], lhsT=fc1wp_t[:, kt, :],
                                 rhs=fusTv[:, kt, :],
                                 start=(kt == 0), stop=(kt == NCH - 1),
                                 skip_group_check=True)

            if PH < 7:
                raise _SkipRest()

            def bn_relu(x_ps, Hdim, g_t, b_t, bni_d, bno_d, tagp):
                xsb = hd.tile([Hdim, GPC], F32, name=f"xsb{tagp}")
                nc.vector.tensor_copy(xsb[:], x_ps[:])
                sums = hd.tile([Hdim, 1], F32, name=f"sums{tagp}")
                nc.vector.reduce_sum(out=sums[:], in_=xsb[:], axis=mybir.AxisListType.X)
                sqj = hd.tile([Hdim, GPC], F32, name=f"sqj{tagp}")
                sumsq = hd.tile([Hdim, 1], F32, name=f"sumsq{tagp}")
                nc.vector.tensor_tensor(out=sqj[:], in0=xsb[:], in1=xsb[:], op=MUL)
                nc.vector.reduce_sum(out=sumsq[:], in_=sqj[:], axis=mybir.AxisListType.X)
                stat = hd.tile([Hdim, 2], F32, name=f"stat{tagp}")
                nc.vector.tensor_copy(stat[:, 0:1], sums[:])
                nc.vector.tensor_copy(stat[:, 1:2], sumsq[:])
                nc.sync.dma_start(bni_d[:], stat[:])
                nc.gpsimd.collective_compute(
                    "AllReduce", ADD, replica_groups=groups,
                    ins=[bni_d[:].opt()], outs=[bno_d[:].opt()])
                statg = hd.tile([Hdim, 2], F32, name=f"statg{tagp}")
                nc.sync.dma_start(statg[:], bno_d[:])
                mean = hd.tile([Hdim, 1], F32, name=f"mean{tagp}")
                nc.vector.tensor_scalar(out=mean[:], in0=statg[:, 0:1],
                                        scalar1=1.0 / B, scalar2=None, op0=MUL)
                var = hd.tile([Hdim, 1], F32, name=f"var{tagp}")
                nc.vector.tensor_scalar(out=var[:], in0=statg[:, 1:2],
                                        scalar1=1.0 / B, scalar2=None, op0=MUL)
                msq = hd.tile([Hdim, 1], F32, name=f"msq{tagp}")
                nc.vector.tensor_tensor(out=msq[:], in0=mean[:], in1=mean[:], op=MUL)
                nc.vector.tensor_tensor(out=var[:], in0=var[:], in1=msq[:], op=SUB)
                nc.vector.tensor_scalar(out=var[:], in0=var[:], scalar1=BN_EPS,
                                        scalar2=None, op0=ADD)
                sd = hd.tile([Hdim, 1], F32, name=f"sd{tagp}")
                nc.scalar.activation(out=sd[:], in_=var[:], func=AF.Sqrt, bias=zcol[:Hdim, :1])
                rsd = hd.tile([Hdim, 1], F32, name=f"rsd{tagp}")
                nc.vector.reciprocal(rsd[:], sd[:])
                scl = hd.tile([Hdim, 1], F32, name=f"scl{tagp}")
                nc.vector.tensor_tensor(out=scl[:], in0=rsd[:], in1=g_t[:], op=MUL)
                tb = hd.tile([Hdim, 1], F32, name=f"tb{tagp}")
                nc.vector.tensor_tensor(out=tb[:], in0=mean[:], in1=scl[:], op=MUL)
                nc.vector.tensor_scalar(out=tb[:], in0=tb[:], scalar1=-1.0,
                                        scalar2=None, op0=MUL)
                nc.vector.tensor_tensor(out=tb[:], in0=tb[:], in1=b_t[:], op=ADD)
                o = hd.tile([Hdim, GPC], F32, name=f"bno{tagp}")
                nc.scalar.activation(out=o[:], in_=xsb[:], func=AF.Relu,
                                     bias=tb[:, 0:1], scale=scl[:, 0:1])
                return o

            bn1o_t = bn_relu(fc1_ps, H1, bn1g_t, bn1b_t, bn1i_d, bn1o_d, "1")
            if PH < 8:
                raise _SkipRest()
            fc2_ps = pp_t1.tile([H2, GPC], F32, tag="t1ps")
            nc.tensor.matmul(fc2_ps[:], lhsT=fc2w_t[:], rhs=bn1o_t[:], start=True, stop=True)
            bn2o_t = bn_relu(fc2_ps, H2, bn2g_t, bn2b_t, bn2i_d, bn2o_d, "2")
            fc3_ps = pp_t1.tile([1, GPC], F32, tag="t1ps")
            nc.tensor.matmul(fc3_ps[:], lhsT=fc3w_t[:], rhs=bn2o_t[:], start=True, stop=True)
            outsb = hd.tile([1, GPC], F32)
            nc.vector.tensor_scalar(out=outsb[:], in0=fc3_ps[:],
                                    scalar1=fc3b_t[0:1, 0:1], scalar2=None, op0=ADD)
            nc.sync.dma_start(out_d[:], outsb[:])
            nc.leave_named_scope("ph6_head", _sc6[0], False)
          except _SkipRest:
            pass

    nc.compile()
    return nc


# ----------------------------------------------------------------------------
# Entry point
# ----------------------------------------------------------------------------

REAL_DIMS = dict(N=50000, E=800000, B=512, DIM_IN=128, GC1=100, DG=20,
                 D2=200, DH=64, H1=128, H2=32)
_CACHE = {}


def run(inputs, nc_cores=8, dims=None, trace=False):
    dims = dims or REAL_DIMS
    meta, per_core = plan(inputs, nc_cores, dims)
    key = repr(sorted(meta.items()))
    if key not in _CACHE:
        _CACHE[key] = build(meta)
    prog = _CACHE[key]
    from concourse.bass_utils import run_bass_kernel_spmd
    res = run_bass_kernel_spmd(prog, per_core, list(range(nc_cores)), trace=trace)
    outs = [np.asarray(res.results[c]["out"]).reshape(-1) for c in range(nc_cores)]
    y = np.concatenate(outs).astype(np.float32)[:, None]
    return y, res


def kernel(**inputs):
    y, _ = run(inputs, nc_cores=8, dims=REAL_DIMS, trace=False)
    return y


# revision 42
# speedup vs baseline: 1.0126x; 1.0126x over previous
"""Trainium2 Bass kernel for nn_BiAttn_TFN_hg_2desc_Net (GNN message passing).

Strategy (8 NeuronCores, SPMD single program):
  - Nodes/graphs sharded by graph (64 graphs/core, contiguous node ranges since
    graph_id is sorted). Edges sharded by dst-owner core.
  - L1 aggregates RAW feat (not feat @ W1): agg1 = A @ feat, then the 128->100
    W1 transform runs per 128-node block AFTER aggregation (8x less matmul
    work than transforming all 50k nodes, no t1 table write, and edge gathers
    start immediately from a host-packed feat table).
  - Edge aggregation: edges bucketed host-side by (dst 128-block, src parity);
    dma_gather (1024 idx/call, the single-packet 64-desc/lane ceiling) fetches
    512B rows (2 nodes) from the packed table, idx = src//2; one-hot selectors
    built in bulk (one bf16 is_equal over [128, 32*128] per 32-tile group via
    a broadcast AP); segment-sum by dst via selector matmuls into PSUM.
  - Block epilogue: hm = agg*rdeg -> PE transpose -> h1 = relu(hmT.T @ W1+b1)
    -> t2 = h1 @ gc2_W via a second PE transpose.
  - One AllGather of the tight t2 ([NPAD,32] bf16) across the 8 cores.
  - L2 aggregation: gather DIRECTLY from the allgathered [TOTPAD, 32] table
    with idx = src_pad//4 (256B rows cover 4 nodes; the rhs slice picks
    sub-row src_pad%4), segment-sum as L1, then graph-mean pooling via
    one-hot graph-selector matmuls.
  - Head: bilinear attention + fusion outer-product + 3-layer MLP with
    BatchNorm, computed feature-major; BN batch stats via two tiny AllReduces.
"""

import sys

sys.path.insert(0, "/opt/trn_rl_repo")

import numpy as np
import ml_dtypes

import concourse.bass as bass
import concourse.bacc as bacc
import concourse.tile as tile
from concourse import mybir
from concourse import bass_utils
from concourse.library_config import mlp as _mlp_lib

bass_utils.upload_artifacts = lambda tmpdir: "local://skipped"

P = 128
TGG = 8         # tiles per dma_gather call (1024 idx; single-packet limit)
TGS = 32        # tiles per bulk selector build
BN_EPS = 1e-5

F32 = mybir.dt.float32
BF16 = mybir.dt.bfloat16
I16 = mybir.dt.int16

BF = ml_dtypes.bfloat16


# ----------------------------------------------------------------------------
# Host-side planning
# ----------------------------------------------------------------------------

def _wrap_idx(flat_idx):
    """[NI] int -> [128, NI//16] int16 in the dma_gather wrapped layout
    (idx i at [i % 16, i // 16], tiled x8 down the partitions)."""
    a = np.asarray(flat_idx, np.int16).reshape(-1, 16).T      # [16, NI/16]
    return np.tile(a, (8, 1))                                  # [128, NI/16]


def plan(inputs, nc_cores, dims):
    """Host preprocessing. Returns (meta, per_core_inputs)."""
    N = dims["N"]; E = dims["E"]; B = dims["B"]
    DIM_IN = dims["DIM_IN"]; GC1 = dims["GC1"]; DG = dims["DG"]
    D2 = dims["D2"]; DH = dims["DH"]; H1 = dims["H1"]; H2 = dims["H2"]
    NC = nc_cores
    GPC = B // NC

    feat = np.asarray(inputs["feat"], np.float32)
    src = np.asarray(inputs["src"], np.int64)
    dst = np.asarray(inputs["dst"], np.int64)
    gid = np.asarray(inputs["graph_id"], np.int64)

    # --- core node/graph ranges (graph-aligned) ---
    bounds = np.searchsorted(gid, np.arange(0, B + 1, GPC))
    g_start, g_end = bounds[:-1], bounds[1:]
    nodes_c = g_end - g_start
    B_blk = int(np.ceil(nodes_c.max() / P))
    NPAD = B_blk * P
    TOTPAD = NC * NPAD
    NROWS = int(np.ceil(N / P)) * P            # feat rows (node space, 128-pad)
    assert NROWS % 2 == 0 and NROWS // 2 < 32768
    assert TOTPAD % 4 == 0 and TOTPAD // 4 < 32768

    # --- degrees / counts ---
    deg = np.bincount(dst, minlength=N).astype(np.float32)
    rdeg_full = 1.0 / np.maximum(deg, 1.0)
    cnt = np.bincount(gid, minlength=B).astype(np.float32)
    rcnt_full = (1.0 / np.maximum(cnt, 1.0)).astype(np.float32)

    # --- edge assignment ---
    core_of_dst = np.searchsorted(g_end - 1, dst)          # g_start <= dst < g_end
    core_of_src = np.searchsorted(g_end - 1, src)
    src_pad = core_of_src * NPAD + (src - g_start[core_of_src])

    # L1 subbuckets: (dst block, src % 2); L2: (dst block, src_pad % 4)
    per_core_edges = []
    cnt1 = np.zeros((NC, B_blk, 2), np.int64)
    cnt2 = np.zeros((NC, B_blk, 4), np.int64)
    for c in range(NC):
        m = core_of_dst == c
        e_src, e_dst, e_srcpad = src[m], dst[m], src_pad[m]
        drel = e_dst - g_start[c]
        blk = drel // P
        drel_in = (drel % P).astype(np.float32)
        i1 = e_src // 2
        p1 = e_src % 2
        i2 = e_srcpad // 4
        q2 = e_srcpad % 4
        buckets = {}
        for b in range(B_blk):
            mb = blk == b
            for p in range(2):
                mm = mb & (p1 == p)
                buckets[("L1", b, p)] = (i1[mm], drel_in[mm])
                cnt1[c, b, p] = mm.sum()
            for q in range(4):
                mm = mb & (q2 == q)
                buckets[("L2", b, q)] = (i2[mm], drel_in[mm])
                cnt2[c, b, q] = mm.sum()
        per_core_edges.append(buckets)

    def _layer_plan(cnt):
        """Dense layout shared across cores: run (b, s) sized to the
        max-over-cores edge count; tiles are 128-chunks of the flat
        concatenation; one selector-matmul per (tile, run) overlap."""
        nsub = cnt.shape[2]
        Lmax = cnt.max(axis=0)                       # [B_blk, nsub]
        for b in range(B_blk):
            if Lmax[b].sum() == 0:
                Lmax[b, 0] = 1                       # keep psum group alive
        off = np.zeros((B_blk, nsub), np.int64)
        tot = 0
        for b in range(B_blk):
            for s in range(nsub):
                off[b, s] = tot
                tot += int(Lmax[b, s])
        NT = -(-tot // P)
        ML = []                                       # (block, tile, sub, selcol)
        Kb = [0] * B_blk
        selc = 0
        for b in range(B_blk):
            for s in range(nsub):
                a0 = int(off[b, s]); ln = int(Lmax[b, s])
                if ln == 0:
                    continue
                for t in range(a0 // P, (a0 + ln - 1) // P + 1):
                    ML.append((b, t, s, selc))
                    Kb[b] += 1
                    selc += 1
        return Lmax, off, tot, NT, tuple(ML), tuple(Kb), selc

    Lmax1, off1, tot1, NT1, ML1, Kb1, NSEL1 = _layer_plan(cnt1)
    Lmax2, off2, tot2, NT2, ML2, Kb2, NSEL2 = _layer_plan(cnt2)
    NG1 = -(-NT1 // TGG); NG2 = -(-NT2 // TGG)
    NSELP1 = -(-NSEL1 // TGS) * TGS
    NSELP2 = -(-NSEL2 // TGS) * TGS

    import os
    meta = dict(
        PH=int(os.environ.get("K_PHASES", "9")),
        NC=NC, B=B, GPC=GPC, B_blk=B_blk, NPAD=NPAD, TOTPAD=TOTPAD,
        NROWS=NROWS,
        ML1=ML1, ML2=ML2, Kb1=Kb1, Kb2=Kb2,
        NT1=NT1, NT2=NT2, NG1=NG1, NG2=NG2,
        NSEL1=NSEL1, NSEL2=NSEL2, NSELP1=NSELP1, NSELP2=NSELP2,
        DIM_IN=DIM_IN, GC1=GC1, DG=DG, D2=D2, DH=DH, H1=H1, H2=H2,
    )

    # --- shared (replicated) tensors ---
    featP = np.zeros((NROWS // 2, 2 * DIM_IN), BF)       # 512B rows, 2 nodes
    featP[:, :] = np.pad(feat, ((0, NROWS - N), (0, 0))).astype(BF).reshape(
        NROWS // 2, 2 * DIM_IN)
    w1 = np.asarray(inputs["gc1_W"], np.float32).astype(BF)          # [128,100]
    w2 = np.asarray(inputs["gc2_W"], np.float32).astype(BF)          # [100,20]
    iota = np.tile(np.arange(P, dtype=np.float32), (P, 1))
    iotaG = np.tile(np.arange(P, dtype=np.float32), (P, TGS)).astype(BF)
    ident = np.eye(P, dtype=np.float32)
    identb = np.eye(P, dtype=np.float32).astype(BF)
    b1b = np.tile(np.asarray(inputs["gc1_b"], np.float32), (P, 1))   # [128,100]
    b2b = np.tile(np.asarray(inputs["gc2_b"], np.float32), (P, 1))   # [128,20]

    pg_W = np.asarray(inputs["pg_W"], np.float32); pg_b = np.asarray(inputs["pg_b"], np.float32)
    p2_W = np.asarray(inputs["p2_W"], np.float32); p2_b = np.asarray(inputs["p2_b"], np.float32)
    W2m = np.asarray(inputs["W2"], np.float32)
    w2eff = np.concatenate([pg_W, pg_b[None, :]], 0) @ W2m            # [21, 64]
    # d1 columns reordered to [1 | a*desc2d] so the ones-column sits at
    # partition 0 of chunk 0 (engines need aligned start partitions).
    p2w_aug = np.concatenate([p2_b[None, :], p2_W], 0)                # [201, 64]
    # p2w in 128-row chunks (chunk 1 zero-padded to 128)
    p2wp = np.zeros((P, 2, DH), np.float32)
    p2wp[:, 0] = p2w_aug[:P]
    p2wp[:D2 + 1 - P, 1] = p2w_aug[P:]
    # fc1_W re-laid for the feature-major fusion: fusion row r = i*(D2+1)+j
    # (j in the reordered space: j=0 -> original 200 (ones), j>0 -> j-1)
    # maps to padded row i*256 + j -> chunk c = 2i + j//128, partition j%128.
    NCH = 2 * (DG + 1)
    fc1wp = np.zeros((P, NCH, H1), np.float32)
    fc1W_r = np.asarray(inputs["fc1_W"], np.float32)
    jperm = np.concatenate([[D2], np.arange(D2)])      # new j -> original j
    for i in range(DG + 1):
        rows = fc1W_r[i * (D2 + 1) + jperm]            # [201, H1] reordered
        for h in range(2):
            jlo = h * P
            jhi = min((h + 1) * P, D2 + 1)
            fc1wp[:jhi - jlo, 2 * i + h] = rows[jlo:jhi]
    fc2w = np.asarray(inputs["fc2_W"], np.float32)
    fc3w = np.asarray(inputs["fc3_W"], np.float32)
    fc3b_r = np.asarray(inputs["fc3_b"], np.float32)
    bn1g = np.asarray(inputs["bn1_g"], np.float32)[:, None]
    bn1b = np.asarray(inputs["bn1_b"], np.float32)[:, None]
    bn2g = np.asarray(inputs["bn2_g"], np.float32)[:, None]
    bn2b = np.asarray(inputs["bn2_b"], np.float32)[:, None]
    # fc biases ride along as [H,1] columns added before BN.
    # BN(x+c) absorbs additive consts into the mean, so fc1_b/fc2_b cancel
    # entirely; fc3_b survives.
    meta["NCH"] = NCH
    desc2d = np.asarray(inputs["desc_2d"], np.float32)                # [B, 200]

    def _flat_plan(buckets, L, Lmax, off, NT, ML, NSELP):
        """Emit flat idx array (dense runs) and per-matmul selector dr
        columns."""
        nsub = Lmax.shape[1]
        idx_flat = np.zeros(NT * P, np.int64)
        occ_dr = np.full(NT * P, 255.0, np.float32)
        for b in range(B_blk):
            for s in range(nsub):
                ii, dd = buckets[(L, b, s)]
                a0 = int(off[b, s])
                idx_flat[a0:a0 + len(ii)] = ii
                occ_dr[a0:a0 + len(ii)] = dd
        drA = np.full(NSELP * P, 255.0, np.float32)
        for (b, t, s, k) in ML:
            a0 = int(off[b, s]); a1 = a0 + int(Lmax[b, s])
            lo = max(t * P, a0); hi = min((t + 1) * P, a1)
            drA[k * P + (lo - t * P):k * P + (hi - t * P)] = occ_dr[lo:hi]
        return idx_flat, drA

    per_core = []
    for c in range(NC):
        buckets = per_core_edges[c]
        i1f, d1f = _flat_plan(buckets, "L1", Lmax1, off1, NT1, ML1, NSELP1)
        i2f, d2f = _flat_plan(buckets, "L2", Lmax2, off2, NT2, ML2, NSELP2)
        NI = TGG * P
        i1f = np.pad(i1f, (0, NG1 * NI - len(i1f)))
        i2f = np.pad(i2f, (0, NG2 * NI - len(i2f)))
        idx1 = np.zeros((NG1, P, NI // 16), np.int16)
        idx2 = np.zeros((NG2, P, NI // 16), np.int16)
        for g in range(NG1):
            idx1[g] = _wrap_idx(i1f[g * NI:(g + 1) * NI])
        for g in range(NG2):
            idx2[g] = _wrap_idx(i2f[g * NI:(g + 1) * NI])
        dr1 = d1f.reshape(NSELP1, P).T.astype(BF).copy()    # [128, NSELP1]
        dr2 = d2f.reshape(NSELP2, P).T.astype(BF).copy()    # [128, NSELP2]

        nloc = int(nodes_c[c])
        rdeg = np.ones((B_blk * P,), np.float32)
        rdeg[:nloc] = rdeg_full[g_start[c]:g_end[c]]
        gidrel = np.full((B_blk * P,), 255.0, np.float32)
        gidrel[:nloc] = (gid[g_start[c]:g_end[c]] - c * GPC).astype(np.float32)
        rcnt = rcnt_full[c * GPC:(c + 1) * GPC][:, None]              # [GPC,1]
        d2c = desc2d[c * GPC:(c + 1) * GPC]                            # [GPC,200]
        d2T_aug = np.concatenate([np.ones((1, GPC), np.float32), d2c.T], 0)  # [201,GPC]
        d2Tp = np.zeros((P, 2, GPC), np.float32)
        d2Tp[:, 0] = d2T_aug[:P]
        d2Tp[:D2 + 1 - P, 1] = d2T_aug[P:]
        per_core.append({
            "featP": featP, "w1": w1, "w2": w2, "iota": iota, "iotaG": iotaG,
            "ident": ident, "identb": identb, "b1b": b1b, "b2b": b2b,
            "idx1": idx1, "dr1": dr1, "idx2": idx2, "dr2": dr2,
            "rdeg": rdeg.reshape(B_blk, P).T.copy(),      # [128, B_blk]
            "gidrel": gidrel.reshape(B_blk, P).T.copy(),  # [128, B_blk]
            "rcnt": rcnt, "d2gm": d2c, "d2Tp": d2Tp,
            "w2eff": w2eff, "p2wp": p2wp,
            "fc1wp": fc1wp, "fc2w": fc2w, "fc3w": fc3w,
            "fc3b": np.array([[float(fc3b_r[0])]], np.float32),
            "bn1g": bn1g, "bn1b": bn1b, "bn2g": bn2g, "bn2b": bn2b,
        })
    return meta, per_core


# ----------------------------------------------------------------------------
# Device program
# ----------------------------------------------------------------------------

def build(meta):
    NC = meta["NC"]; B = meta["B"]; GPC = meta["GPC"]; B_blk = meta["B_blk"]
    NPAD = meta["NPAD"]; TOTPAD = meta["TOTPAD"]
    NROWS = meta["NROWS"]
    ML1 = meta["ML1"]; ML2 = meta["ML2"]; Kb1 = meta["Kb1"]; Kb2 = meta["Kb2"]
    NT1 = meta["NT1"]; NT2 = meta["NT2"]
    NG1 = meta["NG1"]; NG2 = meta["NG2"]
    NSELP1 = meta["NSELP1"]; NSELP2 = meta["NSELP2"]
    DIM_IN = meta["DIM_IN"]; GC1 = meta["GC1"]; DG = meta["DG"]; D2 = meta["D2"]
    DH = meta["DH"]
    H1 = meta["H1"]; H2 = meta["H2"]; NCH = meta["NCH"]; PH = meta["PH"]
    EQ = mybir.AluOpType.is_equal
    MUL = mybir.AluOpType.mult
    ADD = mybir.AluOpType.add
    SUB = mybir.AluOpType.subtract
    AF = mybir.ActivationFunctionType
    NI = TGG * P

    nc = bacc.Bacc("TRN2", target_bir_lowering=False, debug=False, num_devices=NC,
                   num_swdge_queues=2, dynamic_dma_scratch_size=32768)

    def din(name, shape, dt):
        return nc.dram_tensor(name, shape, dt, kind="ExternalInput").ap()

    featP_d = din("featP", [NROWS // 2, 2 * DIM_IN], BF16)
    w1_d = din("w1", [DIM_IN, GC1], BF16)
    w2_d = din("w2", [GC1, DG], BF16)
    iota_d = din("iota", [P, P], F32)
    iotaG_d = din("iotaG", [P, TGS * P], BF16)
    ident_d = din("ident", [P, P], F32)
    identb_d = din("identb", [P, P], BF16)
    b1b_d = din("b1b", [P, GC1], F32)
    b2b_d = din("b2b", [P, DG], F32)
    idx1_d = din("idx1", [NG1, P, NI // 16], I16)
    dr1_d = din("dr1", [P, NSELP1], BF16)
    idx2_d = din("idx2", [NG2, P, NI // 16], I16)
    dr2_d = din("dr2", [P, NSELP2], BF16)
    rdeg_d = din("rdeg", [P, B_blk], F32)
    gidrel_d = din("gidrel", [P, B_blk], F32)
    rcnt_d = din("rcnt", [GPC, 1], F32)
    d2gm_d = din("d2gm", [GPC, D2], F32)
    d2Tp_d = din("d2Tp", [P, 2, GPC], F32)
    w2eff_d = din("w2eff", [DG + 1, 64], F32)
    p2wp_d = din("p2wp", [P, 2, DH], F32)
    fc1wp_d = din("fc1wp", [P, NCH, H1], F32)
    fc2w_d = din("fc2w", [H1, H2], F32)
    fc3w_d = din("fc3w", [H2, 1], F32)
    fc3b_d = din("fc3b", [1, 1], F32)
    bn1g_d = din("bn1g", [H1, 1], F32)
    bn1b_d = din("bn1b", [H1, 1], F32)
    bn2g_d = din("bn2g", [H2, 1], F32)
    bn2b_d = din("bn2b", [H2, 1], F32)

    t2sh_d = nc.dram_tensor("t2shard", [NPAD, 32], BF16).ap()
    t2full_d = nc.dram_tensor("t2full", [TOTPAD, 32], BF16, addr_space="Shared").ap()
    hgrt_d = nc.dram_tensor("hgrt", [1, (DG + 1) * GPC], F32).ap()
    art_d = nc.dram_tensor("art", [1, GPC], F32).ap()
    bn1i_d = nc.dram_tensor("bn1i", [H1, 2], F32).ap()
    bn1o_d = nc.dram_tensor("bn1o", [H1, 2], F32, addr_space="Shared").ap()
    bn2i_d = nc.dram_tensor("bn2i", [H2, 2], F32).ap()
    bn2o_d = nc.dram_tensor("bn2o", [H2, 2], F32, addr_space="Shared").ap()
    out_d = nc.dram_tensor("out", [1, GPC], F32, kind="ExternalOutput").ap()

    groups = [list(range(NC))]

    class _SkipRest(Exception):
        pass

    with tile.TileContext(nc) as tc:
        from contextlib import ExitStack
        with ExitStack() as ctx:
          try:
            cp = ctx.enter_context(tc.tile_pool(name="consts", bufs=1))
            pp_t1 = ctx.enter_context(tc.tile_pool(name="p_t1", bufs=1, space="PSUM"))
            sb_t1 = ctx.enter_context(tc.tile_pool(name="sb_t1", bufs=4))
            ip = ctx.enter_context(tc.tile_pool(name="idx", bufs=4))
            payp = ctx.enter_context(tc.tile_pool(name="pay", bufs=4))
            selp = ctx.enter_context(tc.tile_pool(name="sel", bufs=2))
            drp = ctx.enter_context(tc.tile_pool(name="dr", bufs=2))
            gsp = ctx.enter_context(tc.tile_pool(name="gsel", bufs=4))
            pp_agg = ctx.enter_context(tc.tile_pool(name="p_agg", bufs=2, space="PSUM"))
            pp_tr = ctx.enter_context(tc.tile_pool(name="p_tr", bufs=1, space="PSUM"))
            pp_t2 = ctx.enter_context(tc.tile_pool(name="p_t2", bufs=1, space="PSUM"))
            hpool = ctx.enter_context(tc.tile_pool(name="hwork", bufs=3))
            pp_hg = ctx.enter_context(tc.tile_pool(name="p_hg", bufs=1, space="PSUM"))
            hd = ctx.enter_context(tc.tile_pool(name="head", bufs=1))

            nc.gpsimd.load_library(_mlp_lib)

            # ---- constants ----
            iota_t = cp.tile([P, P], F32); nc.sync.dma_start(iota_t[:], iota_d[:])
            iotaG_t = cp.tile([P, TGS, P], BF16)
            nc.sync.dma_start(iotaG_t[:], iotaG_d[:].rearrange("p (g e) -> p g e", g=TGS))
            zcol = cp.tile([P, 1], F32); nc.vector.memset(zcol[:], 0.0)
            ident_t = cp.tile([P, P], F32); nc.sync.dma_start(ident_t[:], ident_d[:])
            identb_t = cp.tile([P, P], BF16); nc.sync.dma_start(identb_t[:], identb_d[:])
            w1_t = cp.tile([DIM_IN, GC1], BF16); nc.sync.dma_start(w1_t[:], w1_d[:])
            w2_t = cp.tile([GC1, DG], BF16); nc.sync.dma_start(w2_t[:], w2_d[:])
            b1b_t = cp.tile([P, GC1], F32); nc.sync.dma_start(b1b_t[:], b1b_d[:])
            b2b_t = cp.tile([P, DG], F32); nc.sync.dma_start(b2b_t[:], b2b_d[:])
            rdeg_t = cp.tile([P, B_blk], F32); nc.sync.dma_start(rdeg_t[:], rdeg_d[:])
            gidr_t = cp.tile([P, B_blk], F32); nc.sync.dma_start(gidr_t[:], gidrel_d[:])

            # ================= shared edge-layer builder =====================
            def edge_layer(NT, NG, NSELP, ML, Kb, idx_dram, dr_dram, tab_ap,
                           elem, sub_off, ncols, out_block, paytag):
                """Gather calls (TGG tiles, dense) + bulk selectors (TGS
                columns), then segment-sum following the host matmul list."""
                pay_tiles = []
                sel_tiles = []
                for g in range(NG):
                    gt = min(TGG, NT - g * TGG)
                    ni = gt * P
                    ix = ip.tile([P, NI // 16], I16, tag="ix" + paytag)
                    nc.sync.dma_start(ix[:, :ni // 16], idx_dram[g, :, :ni // 16])
                    pay = payp.tile([P, TGG, elem], BF16, tag=paytag)
                    # alternate SWDGE queues: each queue's descriptor
                    # generation runs on its own Q7 core pair (cpu_id/2 ==
                    # queue_num), letting consecutive gathers overlap.
                    nc.gpsimd.dma_gather(pay[:, :gt, :], tab_ap, ix[:, :ni // 16],
                                         ni, ni, elem, queue_num=g % 2)
                    pay_tiles.append(pay)
                for s in range(NSELP // TGS):
                    drG = drp.tile([P, TGS, 1], BF16, tag="dr" + paytag)
                    nc.sync.dma_start(drG[:, :, 0],
                                      dr_dram[:, s * TGS:(s + 1) * TGS])
                    selG = selp.tile([P, TGS, P], BF16, tag="sel" + paytag)
                    nc.vector.tensor_tensor(
                        out=selG[:], in0=iotaG_t[:],
                        in1=drG[:, :, :].broadcast_to([P, TGS, P]), op=EQ)
                    sel_tiles.append(selG)

                cur_b = -1
                ps = None
                ki = 0
                for (b, t, s, k) in ML:
                    if b != cur_b:
                        cur_b = b
                        ps = pp_agg.tile([P, ncols], F32, tag="agg")
                        ki = 0
                    g, off = divmod(t, TGG)
                    sg, soff = divmod(k, TGS)
                    nc.tensor.matmul(
                        ps[:],
                        lhsT=sel_tiles[sg][:, soff, :],
                        rhs=pay_tiles[g][:, off, sub_off * s:sub_off * s + ncols],
                        start=(ki == 0), stop=(ki == Kb[b] - 1))
                    ki += 1
                    if ki == Kb[b]:
                        out_block(b, ps)

            # ---- L1 ----
            def l1_out(b, ps):
                # hm = agg * rdeg (bf16), nodes on partitions
                hm = hpool.tile([P, DIM_IN], BF16, tag="hm")
                nc.vector.tensor_scalar(out=hm[:], in0=ps[:],
                                        scalar1=rdeg_t[:, b:b + 1], scalar2=None,
                                        op0=MUL)
                tp = pp_tr.tile([P, P], BF16, tag="trp")
                nc.tensor.transpose(tp[:], hm[:], identb_t[:])
                hmT = hpool.tile([P, P], BF16, tag="hmT")
                nc.vector.tensor_copy(hmT[:], tp[:])
                h1p = pp_tr.tile([P, GC1], F32, tag="h1p")
                nc.tensor.matmul(h1p[:], lhsT=hmT[:], rhs=w1_t[:], start=True, stop=True)
                h1 = hpool.tile([P, GC1], F32, tag="h1")
                nc.vector.tensor_tensor(out=h1[:], in0=h1p[:], in1=b1b_t[:], op=ADD)
                nc.scalar.activation(out=h1[:], in_=h1[:], func=AF.Relu, bias=zcol[:, :1])
                tp2 = pp_tr.tile([GC1, P], F32, tag="trp2")
                nc.tensor.transpose(tp2[:], h1[:], ident_t[:])
                h1T = hpool.tile([GC1, P], BF16, tag="h1T")
                nc.vector.tensor_copy(h1T[:], tp2[:])
                t2p = pp_t2.tile([P, DG], F32, tag="t2p")
                nc.tensor.matmul(t2p[:], lhsT=h1T[:], rhs=w2_t[:], start=True, stop=True)
                t2s = sb_t1.tile([P, 32], BF16, tag="t2s")
                nc.vector.tensor_copy(t2s[:, :DG], t2p[:])
                nc.vector.memset(t2s[:, DG:], 0.0)
                nc.sync.dma_start(t2sh_d[b * P:(b + 1) * P, :], t2s[:])

            _sc2 = nc.enter_named_scope("ph2_L1", False)
            if PH >= 2:
                edge_layer(NT1, NG1, NSELP1, ML1, Kb1, idx1_d, dr1_d,
                           featP_d[:], 2 * DIM_IN, DIM_IN, DIM_IN, l1_out,
                           "pay1")
            nc.leave_named_scope("ph2_L1", _sc2[0], False)

            # ---- AllGather t2 ----
            _sc3 = nc.enter_named_scope("ph3_ag", False)
            if PH >= 3:
              nc.gpsimd.collective_compute(
                "AllGather", mybir.AluOpType.bypass, replica_groups=groups,
                ins=[t2sh_d[:].opt()], outs=[t2full_d[:].opt()])
            nc.leave_named_scope("ph3_ag", _sc3[0], False)

            # ---- L2 + pooling ----
            do_l2 = PH >= 5
            do_head = PH >= 6
            hg_ps = pp_hg.tile([GPC, DG], F32, tag="hgps")

            def l2_out(b, ps):
                h2t = hpool.tile([P, DG], F32, tag="h2")
                nc.vector.tensor_scalar(out=h2t[:], in0=ps[:],
                                        scalar1=rdeg_t[:, b:b + 1], scalar2=None,
                                        op0=MUL)
                nc.vector.tensor_tensor(out=h2t[:], in0=h2t[:], in1=b2b_t[:], op=ADD)
                nc.scalar.activation(out=h2t[:], in_=h2t[:], func=AF.Relu, bias=zcol[:P, :1])
                selg = gsp.tile([P, GPC], F32, tag="selg")
                nc.vector.tensor_scalar(out=selg[:], in0=iota_t[:, :GPC],
                                        scalar1=gidr_t[:, b:b + 1], scalar2=None,
                                        op0=EQ)
                nc.tensor.matmul(hg_ps[:], lhsT=selg[:], rhs=h2t[:],
                                 start=(b == 0), stop=(b == B_blk - 1),
                                 skip_group_check=True)

            _sc5 = nc.enter_named_scope("ph5_L2", False)
            if do_l2:
                t2view = t2full_d[:].rearrange("(r q) e -> r (q e)", q=4)
                edge_layer(NT2, NG2, NSELP2, ML2, Kb2, idx2_d, dr2_d,
                           t2view, P, 32, DG, l2_out, "pay2")
            nc.leave_named_scope("ph5_L2", _sc5[0], False)

            # ================= Head ==========================================
            if not do_head:
                raise _SkipRest()
            _sc6 = nc.enter_named_scope("ph6_head", False)
            rcnt_t = hd.tile([GPC, 1], F32); nc.sync.dma_start(rcnt_t[:], rcnt_d[:])
            d2gm_t = hd.tile([GPC, D2], F32); nc.sync.dma_start(d2gm_t[:], d2gm_d[:])
            d2Tp_t = hd.tile([P, 2, GPC], F32); nc.sync.dma_start(d2Tp_t[:], d2Tp_d[:])
            w2e_t = hd.tile([DG + 1, 64], F32); nc.sync.dma_start(w2e_t[:], w2eff_d[:])
            p2wp_t = hd.tile([P, 2, DH], F32); nc.sync.dma_start(p2wp_t[:], p2wp_d[:])
            fc1wp_t = hd.tile([P, NCH, H1], F32)
            nc.sync.dma_start(fc1wp_t[:], fc1wp_d[:])
            ones1 = hd.tile([1, P], F32); nc.vector.memset(ones1[:], 1.0)
            fc2w_t = hd.tile([H1, H2], F32); nc.sync.dma_start(fc2w_t[:], fc2w_d[:])
            fc3w_t = hd.tile([H2, 1], F32); nc.sync.dma_start(fc3w_t[:], fc3w_d[:])
            fc3b_t = hd.tile([1, 1], F32); nc.sync.dma_start(fc3b_t[:], fc3b_d[:])
            bn1g_t = hd.tile([H1, 1], F32); nc.sync.dma_start(bn1g_t[:], bn1g_d[:])
            bn1b_t = hd.tile([H1, 1], F32); nc.sync.dma_start(bn1b_t[:], bn1b_d[:])
            bn2g_t = hd.tile([H2, 1], F32); nc.sync.dma_start(bn2g_t[:], bn2g_d[:])
            bn2b_t = hd.tile([H2, 1], F32); nc.sync.dma_start(bn2b_t[:], bn2b_d[:])
            if PH == 60:
                raise _SkipRest()

            # hg1 = [hg | 1]
            hg1 = hd.tile([GPC, DG + 1], F32)
            nc.vector.tensor_scalar(out=hg1[:, :DG], in0=hg_ps[:], scalar1=rcnt_t[:, :1],
                                    scalar2=None, op0=MUL)
            nc.vector.memset(hg1[:, DG:DG + 1], 1.0)
            # hgT
            tp2 = pp_tr.tile([DG + 1, GPC], F32, tag="trp")
            nc.tensor.transpose(tp2[:], hg1[:], ident_t[:GPC, :GPC])
            hgT = hd.tile([DG + 1, GPC], F32)
            nc.vector.tensor_copy(hgT[:], tp2[:])
            # round-trip hgT through DRAM to get it on one partition row
            nc.sync.dma_start(
                hgrt_d[:].rearrange("o (i g) -> (o i) g", i=DG + 1), hgT[:])
            hgflat = hd.tile([1, (DG + 1) * GPC], F32)
            nc.sync.dma_start(hgflat[:], hgrt_d[:])
            # h_gm, h_d (graph-major [GPC, 64])
            hgm_ps = pp_t1.tile([GPC, 64], F32, tag="t1ps")
            nc.tensor.matmul(hgm_ps[:], lhsT=hgT[:], rhs=w2e_t[:], start=True, stop=True)
            hgm_sb = hd.tile([GPC, 64], F32)
            nc.vector.tensor_copy(hgm_sb[:], hgm_ps[:])
            hdm_ps = pp_t1.tile([GPC, 64], F32, tag="t1ps")
            nc.tensor.matmul(hdm_ps[:], lhsT=d2Tp_t[:, 0, :], rhs=p2wp_t[:, 0, :],
                             start=True, stop=False)
            nc.tensor.matmul(hdm_ps[:], lhsT=d2Tp_t[:, 1, :], rhs=p2wp_t[:, 1, :],
                             start=False, stop=True)
            junk = hd.tile([GPC, 64], F32)
            s_t = hd.tile([GPC, 1], F32)
            nc.vector.tensor_tensor(out=junk[:], in0=hgm_sb[:], in1=hdm_ps[:], op=MUL)
            nc.vector.reduce_sum(out=s_t[:], in_=junk[:], axis=mybir.AxisListType.X)
            a_t = hd.tile([GPC, 1], F32)
            nc.scalar.activation(out=a_t[:], in_=s_t[:], func=AF.Sigmoid, bias=zcol[:GPC, :1])
            if PH == 61:
                raise _SkipRest()
            # broadcast a over partitions: DRAM round-trip + ones matmul
            nc.sync.dma_start(art_d[:].rearrange("o (g u) -> (o g) u", u=1), a_t[:])
            aflat = hd.tile([1, GPC], F32)
            nc.sync.dma_start(aflat[:], art_d[:])
            ab_ps = pp_tr.tile([P, GPC], F32, tag="trp")
            nc.tensor.matmul(ab_ps[:], lhsT=ones1[:], rhs=aflat[:],
                             start=True, stop=True)
            aB = hd.tile([P, GPC], F32)
            nc.vector.tensor_copy(aB[:], ab_ps[:])
            # d1Tp = d2Tp * a (ones row restored afterwards)
            d1Tp = hd.tile([P, 2, GPC], F32)
            nc.vector.tensor_tensor(
                out=d1Tp[:], in0=d2Tp_t[:],
                in1=aB[:].rearrange("p (o g) -> p o g", o=1).broadcast_to([P, 2, GPC]),
                op=MUL)
            nc.vector.memset(d1Tp[0:1, 0, :], 1.0)
            # hg1B[p, i, g] = hg1[g, i] replicated across partitions
            # (3 matmul chunks of 7*GPC cols to stay within one PSUM bank)
            hg1B = hd.tile([P, DG + 1, GPC], F32)
            CW = 7 * GPC
            for kk in range(3):
                hgb_ps = pp_tr.tile([P, CW], F32, tag="h1p")
                nc.tensor.matmul(hgb_ps[:], lhsT=ones1[:],
                                 rhs=hgflat[:, kk * CW:(kk + 1) * CW],
                                 start=True, stop=True)
                nc.vector.tensor_copy(
                    hg1B[:, kk * 7:(kk + 1) * 7, :],
                    hgb_ps[:].rearrange("p (i g) -> p i g", i=7))
            # fusT[p, i, h, g] = hg1B[p, i, g] * d1Tp[p, h, g]
            fusT = hd.tile([P, DG + 1, 2, GPC], F32)
            nc.vector.tensor_tensor(
                out=fusT[:],
                in0=hg1B[:].rearrange("p i (o g) -> p i o g", o=1)
                    .broadcast_to([P, DG + 1, 2, GPC]),
                in1=d1Tp[:].rearrange("p (o h) g -> p o h g", o=1)
                    .broadcast_to([P, DG + 1, 2, GPC]),
                op=MUL)
            if PH == 62:
                raise _SkipRest()
            # fc1 (feature-major out [H1, GPC])
            fusTv = fusT[:].rearrange("p i h g -> p (i h) g")
            fc1_ps = pp_t1.tile([H1, GPC], F32, tag="t1ps")
            for kt in range(NCH):
                nc.tensor.matmul(fc1_ps[:], lhsT=fc1wp_t[:, kt, :],
                                 rhs=fusTv[:, kt, :],
                                 start=(kt == 0), stop=(kt == NCH - 1),
                                 skip_group_check=True)

            if PH < 7:
                raise _SkipRest()

            def bn_relu(x_ps, Hdim, g_t, b_t, bni_d, bno_d, tagp):
                xsb = hd.tile([Hdim, GPC], F32, name=f"xsb{tagp}")
                nc.vector.tensor_copy(xsb[:], x_ps[:])
                sums = hd.tile([Hdim, 1], F32, name=f"sums{tagp}")
                nc.vector.reduce_sum(out=sums[:], in_=xsb[:], axis=mybir.AxisListType.X)
                sqj = hd.tile([Hdim, GPC], F32, name=f"sqj{tagp}")
                sumsq = hd.tile([Hdim, 1], F32, name=f"sumsq{tagp}")
                nc.vector.tensor_tensor(out=sqj[:], in0=xsb[:], in1=xsb[:], op=MUL)
                nc.vector.reduce_sum(out=sumsq[:], in_=sqj[:], axis=mybir.AxisListType.X)
                stat = hd.tile([Hdim, 2], F32, name=f"stat{tagp}")
                nc.vector.tensor_copy(stat[:, 0:1], sums[:])
                nc.vector.tensor_copy(stat[:, 1:2], sumsq[:])
                nc.sync.dma_start(bni_d[:], stat[:])
                nc.gpsimd.collective_compute(
                    "AllReduce", ADD, replica_groups=groups,
                    ins=[bni_d[:].opt()], outs=[bno_d[:].opt()])
                statg = hd.tile([Hdim, 2], F32, name=f"statg{tagp}")
                nc.sync.dma_start(statg[:], bno_d[:])
                mean = hd.tile([Hdim, 1], F32, name=f"mean{tagp}")
                nc.vector.tensor_scalar(out=mean[:], in0=statg[:, 0:1],
                                        scalar1=1.0 / B, scalar2=None, op0=MUL)
                var = hd.tile([Hdim, 1], F32, name=f"var{tagp}")
                nc.vector.tensor_scalar(out=var[:], in0=statg[:, 1:2],
                                        scalar1=1.0 / B, scalar2=None, op0=MUL)
                msq = hd.tile([Hdim, 1], F32, name=f"msq{tagp}")
                nc.vector.tensor_tensor(out=msq[:], in0=mean[:], in1=mean[:], op=MUL)
                nc.vector.tensor_tensor(out=var[:], in0=var[:], in1=msq[:], op=SUB)
                nc.vector.tensor_scalar(out=var[:], in0=var[:], scalar1=BN_EPS,
                                        scalar2=None, op0=ADD)
                sd = hd.tile([Hdim, 1], F32, name=f"sd{tagp}")
                nc.scalar.activation(out=sd[:], in_=var[:], func=AF.Sqrt, bias=zcol[:Hdim, :1])
                rsd = hd.tile([Hdim, 1], F32, name=f"rsd{tagp}")
                nc.vector.reciprocal(rsd[:], sd[:])
                scl = hd.tile([Hdim, 1], F32, name=f"scl{tagp}")
                nc.vector.tensor_tensor(out=scl[:], in0=rsd[:], in1=g_t[:], op=MUL)
                tb = hd.tile([Hdim, 1], F32, name=f"tb{tagp}")
                nc.vector.tensor_tensor(out=tb[:], in0=mean[:], in1=scl[:], op=MUL)
                nc.vector.tensor_scalar(out=tb[:], in0=tb[:], scalar1=-1.0,
                                        scalar2=None, op0=MUL)
                nc.vector.tensor_tensor(out=tb[:], in0=tb[:], in1=b_t[:], op=ADD)
                o = hd.tile([Hdim, GPC], F32, name=f"bno{tagp}")
                nc.scalar.activation(out=o[:], in_=xsb[:], func=AF.Relu,
                                     bias=tb[:, 0:1], scale=scl[:, 0:1])
                return o

            bn1o_t = bn_relu(fc1_ps, H1, bn1g_t, bn1b_t, bn1i_d, bn1o_d, "1")
            if PH < 8:
                raise _SkipRest()
            fc2_ps = pp_t1.tile([H2, GPC], F32, tag="t1ps")
            nc.tensor.matmul(fc2_ps[:], lhsT=fc2w_t[:], rhs=bn1o_t[:], start=True, stop=True)
            bn2o_t = bn_relu(fc2_ps, H2, bn2g_t, bn2b_t, bn2i_d, bn2o_d, "2")
            fc3_ps = pp_t1.tile([1, GPC], F32, tag="t1ps")
            nc.tensor.matmul(fc3_ps[:], lhsT=fc3w_t[:], rhs=bn2o_t[:], start=True, stop=True)
            outsb = hd.tile([1, GPC], F32)
            nc.vector.tensor_scalar(out=outsb[:], in0=fc3_ps[:],
                                    scalar1=fc3b_t[0:1, 0:1], scalar2=None, op0=ADD)
            nc.sync.dma_start(out_d[:], outsb[:])
            nc.leave_named_scope("ph6_head", _sc6[0], False)
          except _SkipRest:
            pass

    nc.compile()
    return nc


# ----------------------------------------------------------------------------
# Entry point
# ----------------------------------------------------------------------------

REAL_DIMS = dict(N=50000, E=800000, B=512, DIM_IN=128, GC1=100, DG=20,
                 D2=200, DH=64, H1=128, H2=32)
_CACHE = {}


def run(inputs, nc_cores=8, dims=None, trace=False):
    dims = dims or REAL_DIMS
    meta, per_core = plan(inputs, nc_cores, dims)
    key = repr(sorted(meta.items()))
    if key not in _CACHE:
        _CACHE[key] = build(meta)
    prog = _CACHE[key]
    from concourse.bass_utils import run_bass_kernel_spmd
    res = run_bass_kernel_spmd(prog, per_core, list(range(nc_cores)), trace=trace)
    outs = [np.asarray(res.results[c]["out"]).reshape(-1) for c in range(nc_cores)]
    y = np.concatenate(outs).astype(np.float32)[:, None]
    return y, res


def kernel(**inputs):
    y, _ = run(inputs, nc_cores=8, dims=REAL_DIMS, trace=False)
    return y
